# revision 9
# baseline (speedup 1.0000x reference)
"""Trainium2 Bass kernel for nn_NeuralNetwork_63556926046364 (gnn_message_passing).

The reference computation is a 24-layer sequential MLP chain (indices are
arange, so gather/scatter is just "output of layer l feeds layer l+1"):

    v0 = x                                  # [2048]
    v_{l+1} = silu(W_l v_l + b_l)           # l = 0..22
    out     = W_23 v_23 + b_23              # [2048]

masks are all-ones per the input spec; they are applied exactly on the host
(w = weights * masks) before staging to the device.

Distribution (8 NeuronCores, one trn2 chip): tensor-parallel over each
layer's output rows. Core c owns rows [c*256, (c+1)*256) of every layer.
Per layer: 17 PE matmuls (1 bias row + 16 K-chunks of 128) accumulate the
256-row chunk in PSUM, ScalarE applies silu on the PSUM->SBUF copy, the
1KB chunk bounces to internal DRAM, an 8-rank AllGather rebuilds the full
2048-vector, and a single contiguous DMA lands it in SBUF in the matvec
layout for the next layer. Weights stream from HBM (2MB/layer/core) on a
separate HWDGE ring and are prefetched several layers ahead, fully hidden
under the serial chain.

Contraction layout: K-chunk k of the matvec covers input indices
{p*16 + k : p in 0..127}, i.e. v_sb[p, k] = v[p*16 + k]. This makes the
post-AllGather DMA into SBUF fully contiguous (64B per partition). Host-side
weight pre-arrangement matches: w_prep[c, l, p, k*256+n] = w[l, c*256+n, p*16+k].
"""

import sys

import numpy as np

try:
    import concourse.bass as bass  # noqa: F401
except ImportError:  # pragma: no cover - harness env should match dev env
    sys.path.extend([
        "/root/.axon_site",
        "/root/.axon_site/_ro/trn_rl_repo",
        "/root/.axon_site/_ro/pypackages",
    ])

import concourse.bass as bass
import concourse.mybir as mybir
import concourse.tile as tile
from concourse import bacc
from concourse.bass import ts
from concourse.bass import _add_dep_helper
from concourse.bass_utils import run_bass_kernel_spmd

N_LAYERS = 24
WIDTH = 2048
N_CORES = 8
CHUNK = WIDTH // N_CORES          # 256 output rows per core per layer
K_CHUNKS = WIDTH // 128           # 16 contraction chunks of 128

# matmul operand dtype: float32r is fp32 storage with a reduced-precision
# (full-rate) PE multiply; plain float32 streams at 1/4 rate.
USE_F32R = True

# Dummy (zero) bf16 matmuls issued between layers to keep the PE HAM clock
# at 2.4 GHz across the AllGather gaps (PE idles ~7us/layer otherwise and
# re-throttles to 1.2 GHz). Each is ~213ns warm.
DUMMY_MM_PER_LAYER = 12

# Weight DMA for layer l is gated on the completion of layer l-1's
# AllGather, so the 2MB burst runs during the matmul/activation window and
# is finished before the next AllGather's data phase needs the SDMA engines.
W_GATE_LAG = 1

_CACHE = {}


def _build():
    """Build + compile the SPMD Bass program (one program, 8 cores)."""
    if "nc" in _CACHE:
        return _CACHE["nc"]

    f32 = mybir.dt.float32
    f32r = mybir.dt.float32r if USE_F32R else mybir.dt.float32

    nc = bacc.Bacc(
        "TRN2",
        target_bir_lowering=False,
        debug=False,
        enable_asserts=False,
        num_devices=N_CORES,
    )

    w_dram = nc.dram_tensor("w", [N_LAYERS, 128, K_CHUNKS * CHUNK], f32r,
                            kind="ExternalInput")
    bias_dram = nc.dram_tensor("bias", [1, N_LAYERS * CHUNK], f32r,
                               kind="ExternalInput")
    x0_dram = nc.dram_tensor("x0", [128, K_CHUNKS], f32r, kind="ExternalInput")
    one_dram = nc.dram_tensor("one", [1, 1], f32r, kind="ExternalInput")
    out_dram = nc.dram_tensor("out", [1, CHUNK], f32, kind="ExternalOutput")

    with tile.TileContext(nc) as tc:
        with tc.tile_pool(name="const", bufs=1) as const_pool, \
             tc.tile_pool(name="wpool", bufs=6) as wpool, \
             tc.tile_pool(name="vpool", bufs=2) as vpool, \
             tc.tile_pool(name="apool", bufs=2) as apool, \
             tc.tile_pool(name="psum", bufs=2, space="PSUM") as psum_pool, \
             tc.tile_pool(name="ccin", bufs=2, space="DRAM") as ccin_pool, \
             tc.tile_pool(name="ccout", bufs=2, space="DRAM") as ccout_pool:

            bias_sb = const_pool.tile([1, N_LAYERS * CHUNK], f32r)
            nc.sync.dma_start(out=bias_sb[:], in_=bias_dram[:])
            one_sb = const_pool.tile([1, 1], f32r)
            nc.sync.dma_start(out=one_sb[:], in_=one_dram[:])

            bf16 = mybir.dt.bfloat16
            dummy_sb = const_pool.tile([128, 512], bf16)
            nc.vector.memset(dummy_sb[:], 0.0)

            v_sb = vpool.tile([128, K_CHUNKS], f32r, name="v_sb")
            nc.sync.dma_start(out=v_sb[:], in_=x0_dram[:])

            cc_in_dmas = []
            collectives = []
            for l in range(N_LAYERS):
                w_sb = wpool.tile([128, K_CHUNKS * CHUNK], f32r, name="w_sb")
                w_dma = nc.scalar.dma_start(out=w_sb[:], in_=w_dram[l])
                if l >= W_GATE_LAG and collectives:
                    _add_dep_helper(
                        w_dma.ins, collectives[l - W_GATE_LAG].ins, sync=True,
                        reason="stage weight burst outside the AllGather data phase")

                psum = psum_pool.tile([1, CHUNK], f32, name="psum")
                # bias row: psum[0, :] = 1.0 * bias[l*CHUNK:]
                nc.tensor.matmul(psum[:], one_sb[:],
                                 bias_sb[:, ts(l, CHUNK)],
                                 start=True, stop=False)
                for k in range(K_CHUNKS):
                    nc.tensor.matmul(psum[:], v_sb[:, k:k + 1],
                                     w_sb[:, ts(k, CHUNK)],
                                     start=False, stop=(k == K_CHUNKS - 1))

                last = l == N_LAYERS - 1
                act_sb = apool.tile([1, CHUNK], f32 if last else f32r,
                                    name="act_sb")
                func = (mybir.ActivationFunctionType.Copy if last
                        else mybir.ActivationFunctionType.Silu)
                nc.scalar.activation(act_sb[:], psum[:], func)

                if last:
                    nc.sync.dma_start(out=out_dram[:], in_=act_sb[:])
                    break

                cc_in = ccin_pool.tile([1, CHUNK], f32r, name="cc_in")
                cc_in_dma = nc.sync.dma_start(out=cc_in[:], in_=act_sb[:],
                                              single_packet=True)
                cc_in_dmas.append(cc_in_dma)
                cc_out = ccout_pool.tile([1, WIDTH], f32r, name="cc_out",
                                         addr_space="Shared")
                cc = nc.gpsimd.collective_compute(
                    "AllGather",
                    mybir.AluOpType.bypass,
                    replica_groups=[list(range(N_CORES))],
                    ins=[cc_in.opt()],
                    outs=[cc_out.opt()],
                )
                collectives.append(cc)
                # keep the PE busy across the AllGather gap (HAM stays warm)
                for _ in range(DUMMY_MM_PER_LAYER):
                    psum_d = psum_pool.tile([1, 512], f32, name="psum_d",
                                            tag="psum_d", bufs=1)
                    nc.tensor.matmul(psum_d[:], dummy_sb[:, 0:1], dummy_sb[:],
                                     start=True, stop=True)
                v_sb = vpool.tile([128, K_CHUNKS], f32r, name="v_sb")
                nc.sync.dma_start(
                    out=v_sb[:],
                    in_=cc_out.rearrange("a (p k) -> (a p) k", k=K_CHUNKS),
                )

    nc.compile()
    _CACHE["nc"] = nc
    return nc


def _prep_inputs(x, weights, masks, biases):
    """Host-side: apply masks exactly, shard + rearrange into device layout."""
    w = (np.asarray(weights, dtype=np.float32)
         * np.asarray(masks, dtype=np.float32))
    x = np.asarray(x, dtype=np.float32)
    biases = np.asarray(biases, dtype=np.float32)

    # w_prep[c, l, p, k*CHUNK+n] = w[l, c*CHUNK+n, p*16+k]
    w4 = w.reshape(N_LAYERS, N_CORES, CHUNK, 128, K_CHUNKS)   # l c n p k
    w_prep = np.ascontiguousarray(w4.transpose(1, 0, 3, 4, 2)).reshape(
        N_CORES, N_LAYERS, 128, K_CHUNKS * CHUNK)

    x0 = np.ascontiguousarray(x.reshape(128, K_CHUNKS))
    one = np.ones((1, 1), dtype=np.float32)

    in_maps = []
    for c in range(N_CORES):
        bias_c = np.ascontiguousarray(
            biases[:, c * CHUNK:(c + 1) * CHUNK]).reshape(1, N_LAYERS * CHUNK)
        in_maps.append({
            "w": w_prep[c],
            "bias": bias_c,
            "x0": x0,
            "one": one,
        })
    return in_maps


def _install_ntff_hook():
    """The image's antenv package lacks axon_hooks; synthesize it and register
    the ctypes NTFF profiling hook so trace=True yields exec_time_ns."""
    import types

    try:
        from antenv.axon_hooks import get_axon_ntff_profile_hook  # noqa: F401
        return  # already present
    except ImportError:
        pass

    import antenv
    from trn_agent_boot.trn_boot import _ntff_profile_via_ctypes

    mod = types.ModuleType("antenv.axon_hooks")
    _state = {"hook": None}

    def set_axon_ntff_profile_hook(hook):
        _state["hook"] = hook

    def get_axon_ntff_profile_hook():
        return _state["hook"]

    mod.set_axon_ntff_profile_hook = set_axon_ntff_profile_hook
    mod.get_axon_ntff_profile_hook = get_axon_ntff_profile_hook
    sys.modules["antenv.axon_hooks"] = mod
    antenv.axon_hooks = mod
    set_axon_ntff_profile_hook(
        _ntff_profile_via_ctypes("/opt/axon/libaxon_pjrt.so"))


def _run(in_maps, trace=False):
    nc = _build()
    if trace:
        _install_ntff_hook()
    return run_bass_kernel_spmd(nc, in_maps, core_ids=list(range(N_CORES)),
                                trace=trace)


def kernel(x, weights, masks, biases, indices):
    in_maps = _prep_inputs(x, weights, masks, biases)
    res = _run(in_maps)
    out = np.concatenate(
        [np.asarray(res.results[c]["out"][0], dtype=np.float32)
         for c in range(N_CORES)])
    return out


def bench(x, weights, masks, biases, indices):
    """Traced run: returns (output, exec_time_ns)."""
    in_maps = _prep_inputs(x, weights, masks, biases)
    res = _run(in_maps, trace=True)
    out = np.concatenate(
        [np.asarray(res.results[c]["out"][0], dtype=np.float32)
         for c in range(N_CORES)])
    return out, res.exec_time_ns


# revision 10
# speedup vs baseline: 1.0672x; 1.0672x over previous
"""Trainium2 Bass kernel for nn_NeuralNetwork_63556926046364 (gnn_message_passing).

The reference computation is a 24-layer sequential MLP chain (indices are
arange, so gather/scatter is just "output of layer l feeds layer l+1"):

    v0 = x                                  # [2048]
    v_{l+1} = silu(W_l v_l + b_l)           # l = 0..22
    out     = W_23 v_23 + b_23              # [2048]

masks are all-ones per the input spec; they are applied exactly on the host
(w = weights * masks) before staging to the device.

Distribution (8 NeuronCores, one trn2 chip): tensor-parallel over each
layer's output rows. Core c owns rows [c*256, (c+1)*256) of every layer.
Per layer: 17 PE matmuls (1 bias row + 16 K-chunks of 128) accumulate the
256-row chunk in PSUM, ScalarE applies silu on the PSUM->SBUF copy, the
1KB chunk bounces to internal DRAM, an 8-rank AllGather rebuilds the full
2048-vector, and a single contiguous DMA lands it in SBUF in the matvec
layout for the next layer. Weights stream from HBM (2MB/layer/core) on a
separate HWDGE ring and are prefetched several layers ahead, fully hidden
under the serial chain.

Contraction layout: K-chunk k of the matvec covers input indices
{p*16 + k : p in 0..127}, i.e. v_sb[p, k] = v[p*16 + k]. This makes the
post-AllGather DMA into SBUF fully contiguous (64B per partition). Host-side
weight pre-arrangement matches: w_prep[c, l, p, k*256+n] = w[l, c*256+n, p*16+k].
"""

import sys

import numpy as np

try:
    import concourse.bass as bass  # noqa: F401
except ImportError:  # pragma: no cover - harness env should match dev env
    sys.path.extend([
        "/root/.axon_site",
        "/root/.axon_site/_ro/trn_rl_repo",
        "/root/.axon_site/_ro/pypackages",
    ])

import concourse.bass as bass
import concourse.mybir as mybir
import concourse.tile as tile
from concourse import bacc
from concourse.bass import ts
from concourse.bass import _add_dep_helper
from concourse.bass_utils import run_bass_kernel_spmd

N_LAYERS = 24
WIDTH = 2048
N_CORES = 8
CHUNK = WIDTH // N_CORES          # 256 output rows per core per layer
K_CHUNKS = WIDTH // 128           # 16 contraction chunks of 128

# matmul operand dtype: float32r is fp32 storage with a reduced-precision
# (full-rate) PE multiply; plain float32 streams at 1/4 rate.
USE_F32R = True

# Dummy (zero) bf16 matmuls issued between layers to keep the PE HAM clock
# at 2.4 GHz across the AllGather gaps (PE idles ~7us/layer otherwise and
# re-throttles to 1.2 GHz). Each is ~213ns warm.
DUMMY_MM_PER_LAYER = 12

# Weight DMA for layer l is gated on the completion of layer l-1's
# AllGather, so the 2MB burst runs during the matmul/activation window and
# is finished before the next AllGather's data phase needs the SDMA engines.
W_GATE_LAG = 2

_CACHE = {}


def _build():
    """Build + compile the SPMD Bass program (one program, 8 cores)."""
    if "nc" in _CACHE:
        return _CACHE["nc"]

    f32 = mybir.dt.float32
    f32r = mybir.dt.float32r if USE_F32R else mybir.dt.float32

    nc = bacc.Bacc(
        "TRN2",
        target_bir_lowering=False,
        debug=False,
        enable_asserts=False,
        num_devices=N_CORES,
    )

    w_dram = nc.dram_tensor("w", [N_LAYERS, 128, K_CHUNKS * CHUNK], f32r,
                            kind="ExternalInput")
    bias_dram = nc.dram_tensor("bias", [1, N_LAYERS * CHUNK], f32r,
                               kind="ExternalInput")
    x0_dram = nc.dram_tensor("x0", [128, K_CHUNKS], f32r, kind="ExternalInput")
    one_dram = nc.dram_tensor("one", [1, 1], f32r, kind="ExternalInput")
    out_dram = nc.dram_tensor("out", [1, CHUNK], f32, kind="ExternalOutput")

    with tile.TileContext(nc) as tc:
        with tc.tile_pool(name="const", bufs=1) as const_pool, \
             tc.tile_pool(name="wpool", bufs=6) as wpool, \
             tc.tile_pool(name="vpool", bufs=2) as vpool, \
             tc.tile_pool(name="apool", bufs=2) as apool, \
             tc.tile_pool(name="psum", bufs=2, space="PSUM") as psum_pool, \
             tc.tile_pool(name="ccin", bufs=2, space="DRAM") as ccin_pool, \
             tc.tile_pool(name="ccout", bufs=2, space="DRAM") as ccout_pool:

            bias_sb = const_pool.tile([1, N_LAYERS * CHUNK], f32r)
            nc.sync.dma_start(out=bias_sb[:], in_=bias_dram[:])
            one_sb = const_pool.tile([1, 1], f32r)
            nc.sync.dma_start(out=one_sb[:], in_=one_dram[:])

            bf16 = mybir.dt.bfloat16
            dummy_sb = const_pool.tile([128, 512], bf16)
            nc.vector.memset(dummy_sb[:], 0.0)

            v_sb = vpool.tile([128, K_CHUNKS], f32r, name="v_sb")
            nc.sync.dma_start(out=v_sb[:], in_=x0_dram[:])

            cc_in_dmas = []
            collectives = []
            for l in range(N_LAYERS):
                w_sb = wpool.tile([128, K_CHUNKS * CHUNK], f32r, name="w_sb")
                w_dma = nc.scalar.dma_start(out=w_sb[:], in_=w_dram[l])
                if l >= W_GATE_LAG and collectives:
                    _add_dep_helper(
                        w_dma.ins, collectives[l - W_GATE_LAG].ins, sync=True,
                        reason="stage weight burst outside the AllGather data phase")

                psum = psum_pool.tile([1, CHUNK], f32, name="psum")
                # bias row: psum[0, :] = 1.0 * bias[l*CHUNK:]
                nc.tensor.matmul(psum[:], one_sb[:],
                                 bias_sb[:, ts(l, CHUNK)],
                                 start=True, stop=False)
                for k in range(K_CHUNKS):
                    nc.tensor.matmul(psum[:], v_sb[:, k:k + 1],
                                     w_sb[:, ts(k, CHUNK)],
                                     start=False, stop=(k == K_CHUNKS - 1))

                last = l == N_LAYERS - 1
                act_sb = apool.tile([1, CHUNK], f32 if last else f32r,
                                    name="act_sb")
                func = (mybir.ActivationFunctionType.Copy if last
                        else mybir.ActivationFunctionType.Silu)
                nc.scalar.activation(act_sb[:], psum[:], func)

                if last:
                    nc.sync.dma_start(out=out_dram[:], in_=act_sb[:])
                    break

                cc_in = ccin_pool.tile([1, CHUNK], f32r, name="cc_in")
                cc_in_dma = nc.sync.dma_start(out=cc_in[:], in_=act_sb[:],
                                              single_packet=True)
                cc_in_dmas.append(cc_in_dma)
                cc_out = ccout_pool.tile([1, WIDTH], f32r, name="cc_out",
                                         addr_space="Shared")
                cc = nc.gpsimd.collective_compute(
                    "AllGather",
                    mybir.AluOpType.bypass,
                    replica_groups=[list(range(N_CORES))],
                    ins=[cc_in.opt()],
                    outs=[cc_out.opt()],
                )
                collectives.append(cc)
                # keep the PE busy across the AllGather gap (HAM stays warm)
                for _ in range(DUMMY_MM_PER_LAYER):
                    psum_d = psum_pool.tile([1, 512], f32, name="psum_d",
                                            tag="psum_d", bufs=1)
                    nc.tensor.matmul(psum_d[:], dummy_sb[:, 0:1], dummy_sb[:],
                                     start=True, stop=True)
                v_sb = vpool.tile([128, K_CHUNKS], f32r, name="v_sb")
                nc.sync.dma_start(
                    out=v_sb[:],
                    in_=cc_out.rearrange("a (p k) -> (a p) k", k=K_CHUNKS),
                )

    nc.compile()
    _CACHE["nc"] = nc
    return nc


def _prep_inputs(x, weights, masks, biases):
    """Host-side: apply masks exactly, shard + rearrange into device layout."""
    w = (np.asarray(weights, dtype=np.float32)
         * np.asarray(masks, dtype=np.float32))
    x = np.asarray(x, dtype=np.float32)
    biases = np.asarray(biases, dtype=np.float32)

    # w_prep[c, l, p, k*CHUNK+n] = w[l, c*CHUNK+n, p*16+k]
    w4 = w.reshape(N_LAYERS, N_CORES, CHUNK, 128, K_CHUNKS)   # l c n p k
    w_prep = np.ascontiguousarray(w4.transpose(1, 0, 3, 4, 2)).reshape(
        N_CORES, N_LAYERS, 128, K_CHUNKS * CHUNK)

    x0 = np.ascontiguousarray(x.reshape(128, K_CHUNKS))
    one = np.ones((1, 1), dtype=np.float32)

    in_maps = []
    for c in range(N_CORES):
        bias_c = np.ascontiguousarray(
            biases[:, c * CHUNK:(c + 1) * CHUNK]).reshape(1, N_LAYERS * CHUNK)
        in_maps.append({
            "w": w_prep[c],
            "bias": bias_c,
            "x0": x0,
            "one": one,
        })
    return in_maps


def _install_ntff_hook():
    """The image's antenv package lacks axon_hooks; synthesize it and register
    the ctypes NTFF profiling hook so trace=True yields exec_time_ns."""
    import types

    try:
        from antenv.axon_hooks import get_axon_ntff_profile_hook  # noqa: F401
        return  # already present
    except ImportError:
        pass

    import antenv
    from trn_agent_boot.trn_boot import _ntff_profile_via_ctypes

    mod = types.ModuleType("antenv.axon_hooks")
    _state = {"hook": None}

    def set_axon_ntff_profile_hook(hook):
        _state["hook"] = hook

    def get_axon_ntff_profile_hook():
        return _state["hook"]

    mod.set_axon_ntff_profile_hook = set_axon_ntff_profile_hook
    mod.get_axon_ntff_profile_hook = get_axon_ntff_profile_hook
    sys.modules["antenv.axon_hooks"] = mod
    antenv.axon_hooks = mod
    set_axon_ntff_profile_hook(
        _ntff_profile_via_ctypes("/opt/axon/libaxon_pjrt.so"))


def _run(in_maps, trace=False):
    nc = _build()
    if trace:
        _install_ntff_hook()
    return run_bass_kernel_spmd(nc, in_maps, core_ids=list(range(N_CORES)),
                                trace=trace)


def kernel(x, weights, masks, biases, indices):
    in_maps = _prep_inputs(x, weights, masks, biases)
    res = _run(in_maps)
    out = np.concatenate(
        [np.asarray(res.results[c]["out"][0], dtype=np.float32)
         for c in range(N_CORES)])
    return out


def bench(x, weights, masks, biases, indices):
    """Traced run: returns (output, exec_time_ns)."""
    in_maps = _prep_inputs(x, weights, masks, biases)
    res = _run(in_maps, trace=True)
    out = np.concatenate(
        [np.asarray(res.results[c]["out"][0], dtype=np.float32)
         for c in range(N_CORES)])
    return out, res.exec_time_ns


# revision 11
# speedup vs baseline: 1.2719x; 1.1919x over previous
"""Trainium2 Bass kernel for nn_NeuralNetwork_63556926046364 (gnn_message_passing).

The reference computation is a 24-layer sequential MLP chain (indices are
arange, so gather/scatter is just "output of layer l feeds layer l+1"):

    v0 = x                                  # [2048]
    v_{l+1} = silu(W_l v_l + b_l)           # l = 0..22
    out     = W_23 v_23 + b_23              # [2048]

masks are all-ones per the input spec; they are applied exactly on the host
(w = weights * masks) before staging to the device.

Distribution (8 NeuronCores, one trn2 chip): tensor-parallel over each
layer's output rows. Core c owns rows [c*256, (c+1)*256) of every layer.
Per layer: 17 PE matmuls (1 bias row + 16 K-chunks of 128) accumulate the
256-row chunk in PSUM, ScalarE applies silu on the PSUM->SBUF copy, the
1KB chunk bounces to internal DRAM, an 8-rank AllGather rebuilds the full
2048-vector, and a single contiguous DMA lands it in SBUF in the matvec
layout for the next layer. Weights stream from HBM (2MB/layer/core) on a
separate HWDGE ring and are prefetched several layers ahead, fully hidden
under the serial chain.

Contraction layout: K-chunk k of the matvec covers input indices
{p*16 + k : p in 0..127}, i.e. v_sb[p, k] = v[p*16 + k]. This makes the
post-AllGather DMA into SBUF fully contiguous (64B per partition). Host-side
weight pre-arrangement matches: w_prep[c, l, p, k*256+n] = w[l, c*256+n, p*16+k].
"""

import sys

import numpy as np

try:
    import concourse.bass as bass  # noqa: F401
except ImportError:  # pragma: no cover - harness env should match dev env
    sys.path.extend([
        "/root/.axon_site",
        "/root/.axon_site/_ro/trn_rl_repo",
        "/root/.axon_site/_ro/pypackages",
    ])

import concourse.bass as bass
import concourse.mybir as mybir
import concourse.tile as tile
from concourse import bacc
from concourse.bass import ts
from concourse.bass import _add_dep_helper
from concourse.bass_utils import run_bass_kernel_spmd

N_LAYERS = 24
WIDTH = 2048
N_CORES = 8
CHUNK = WIDTH // N_CORES          # 256 output rows per core per layer
K_CHUNKS = WIDTH // 128           # 16 contraction chunks of 128

# matmul operand dtype:
#  - "f32r": fp32 storage, reduced-precision full-rate PE multiply (~1.3e-4
#    end-to-end rel err). 2MB/layer/core weight stream.
#  - "bf16": half the weight stream (1MB/layer/core), same PE rate, larger
#    rounding error (measure!).
#  - "f32": exact but PE streams at 1/4 rate.
MM_DTYPE = "f32r"

# Dummy (zero) bf16 matmuls issued between layers to keep the PE HAM clock
# at 2.4 GHz across the AllGather gaps (PE idles ~7us/layer otherwise and
# re-throttles to 1.2 GHz). Each is ~213ns warm.
DUMMY_MM_PER_LAYER = 14

# Weight DMA for layer l is gated on the completion of layer l-1's
# AllGather, so the 2MB burst runs during the matmul/activation window and
# is finished before the next AllGather's data phase needs the SDMA engines.
W_GATE_LAG = 2

_CACHE = {}


def _build():
    """Build + compile the SPMD Bass program (one program, 8 cores)."""
    if "nc" in _CACHE:
        return _CACHE["nc"]

    f32 = mybir.dt.float32
    f32r = {"f32r": mybir.dt.float32r, "bf16": mybir.dt.bfloat16,
            "f32": mybir.dt.float32}[MM_DTYPE]

    nc = bacc.Bacc(
        "TRN2",
        target_bir_lowering=False,
        debug=False,
        enable_asserts=False,
        num_devices=N_CORES,
    )

    w_dram = nc.dram_tensor("w", [N_LAYERS, 128, K_CHUNKS * CHUNK], f32r,
                            kind="ExternalInput")
    bias_dram = nc.dram_tensor("bias", [1, N_LAYERS * CHUNK], f32r,
                               kind="ExternalInput")
    x0_dram = nc.dram_tensor("x0", [128, K_CHUNKS], f32r, kind="ExternalInput")
    one_dram = nc.dram_tensor("one", [1, 1], f32r, kind="ExternalInput")
    out_dram = nc.dram_tensor("out", [1, CHUNK], f32, kind="ExternalOutput")

    with tile.TileContext(nc) as tc:
        with tc.tile_pool(name="const", bufs=1) as const_pool, \
             tc.tile_pool(name="wpool", bufs=6) as wpool, \
             tc.tile_pool(name="vpool", bufs=2) as vpool, \
             tc.tile_pool(name="apool", bufs=2) as apool, \
             tc.tile_pool(name="psum", bufs=2, space="PSUM") as psum_pool, \
             tc.tile_pool(name="ccin", bufs=2, space="DRAM") as ccin_pool, \
             tc.tile_pool(name="ccout", bufs=2, space="DRAM") as ccout_pool:

            bias_sb = const_pool.tile([1, N_LAYERS * CHUNK], f32r)
            nc.sync.dma_start(out=bias_sb[:], in_=bias_dram[:])
            one_sb = const_pool.tile([1, 1], f32r)
            nc.sync.dma_start(out=one_sb[:], in_=one_dram[:])

            bf16 = mybir.dt.bfloat16
            dummy_sb = const_pool.tile([128, 512], bf16)
            nc.vector.memset(dummy_sb[:], 0.0)

            v_sb = vpool.tile([128, K_CHUNKS], f32r, name="v_sb")
            nc.sync.dma_start(out=v_sb[:], in_=x0_dram[:])

            cc_in_dmas = []
            collectives = []
            v_loads = []
            for l in range(N_LAYERS):
                w_sb = wpool.tile([128, K_CHUNKS * CHUNK], f32r, name="w_sb")
                w_dma = nc.scalar.dma_start(out=w_sb[:], in_=w_dram[l])
                if l >= W_GATE_LAG and v_loads:
                    _add_dep_helper(
                        w_dma.ins, v_loads[l - W_GATE_LAG].ins, sync=True,
                        reason="stage weight burst into the matmul window")

                psum = psum_pool.tile([1, CHUNK], f32, name="psum")
                # bias row: psum[0, :] = 1.0 * bias[l*CHUNK:]
                nc.tensor.matmul(psum[:], one_sb[:],
                                 bias_sb[:, ts(l, CHUNK)],
                                 start=True, stop=False)
                for k in range(K_CHUNKS):
                    nc.tensor.matmul(psum[:], v_sb[:, k:k + 1],
                                     w_sb[:, ts(k, CHUNK)],
                                     start=False, stop=(k == K_CHUNKS - 1))

                last = l == N_LAYERS - 1
                act_sb = apool.tile([1, CHUNK], f32 if last else f32r,
                                    name="act_sb")
                func = (mybir.ActivationFunctionType.Copy if last
                        else mybir.ActivationFunctionType.Silu)
                act = nc.scalar.activation(act_sb[:], psum[:], func)

                if last:
                    nc.sync.dma_start(out=out_dram[:], in_=act_sb[:])
                    break

                cc_in = ccin_pool.tile([1, CHUNK], f32r, name="cc_in")
                cc_in_dma = nc.sync.dma_start(out=cc_in[:], in_=act_sb[:],
                                              single_packet=True)
                cc_in_dmas.append(cc_in_dma)
                cc_out = ccout_pool.tile([1, WIDTH], f32r, name="cc_out",
                                         addr_space="Shared")
                cc = nc.gpsimd.collective_compute(
                    "AllGather",
                    mybir.AluOpType.bypass,
                    replica_groups=[list(range(N_CORES))],
                    ins=[cc_in.opt()],
                    outs=[cc_out.opt()],
                )
                collectives.append(cc)
                # keep the PE busy across the AllGather gap (HAM stays warm);
                # chained to this layer's SILU so the scheduler can't hoist
                # the whole block to kernel start.
                for d in range(DUMMY_MM_PER_LAYER):
                    psum_d = psum_pool.tile([1, 512], f32, name="psum_d",
                                            tag="psum_d", bufs=1)
                    dmm = nc.tensor.matmul(psum_d[:], dummy_sb[:, 0:1],
                                           dummy_sb[:], start=True, stop=True)
                    if d == 0:
                        _add_dep_helper(
                            dmm.ins, act.ins, sync=True,
                            reason="dummy warmup block follows this layer's silu")
                v_sb = vpool.tile([128, K_CHUNKS], f32r, name="v_sb")
                v_load = nc.sync.dma_start(
                    out=v_sb[:],
                    in_=cc_out.rearrange("a (p k) -> (a p) k", k=K_CHUNKS),
                )
                v_loads.append(v_load)

    nc.compile()
    _CACHE["nc"] = nc
    return nc


def _prep_inputs(x, weights, masks, biases):
    """Host-side: apply masks exactly, shard + rearrange into device layout."""
    w = (np.asarray(weights, dtype=np.float32)
         * np.asarray(masks, dtype=np.float32))
    x = np.asarray(x, dtype=np.float32)
    biases = np.asarray(biases, dtype=np.float32)

    # w_prep[c, l, p, k*CHUNK+n] = w[l, c*CHUNK+n, p*16+k]
    w4 = w.reshape(N_LAYERS, N_CORES, CHUNK, 128, K_CHUNKS)   # l c n p k
    w_prep = np.ascontiguousarray(w4.transpose(1, 0, 3, 4, 2)).reshape(
        N_CORES, N_LAYERS, 128, K_CHUNKS * CHUNK)

    x0 = np.ascontiguousarray(x.reshape(128, K_CHUNKS))
    one = np.ones((1, 1), dtype=np.float32)

    in_maps = []
    for c in range(N_CORES):
        bias_c = np.ascontiguousarray(
            biases[:, c * CHUNK:(c + 1) * CHUNK]).reshape(1, N_LAYERS * CHUNK)
        in_maps.append({
            "w": w_prep[c],
            "bias": bias_c,
            "x0": x0,
            "one": one,
        })
    return in_maps


def _install_ntff_hook():
    """The image's antenv package lacks axon_hooks; synthesize it and register
    the ctypes NTFF profiling hook so trace=True yields exec_time_ns."""
    import types

    try:
        from antenv.axon_hooks import get_axon_ntff_profile_hook  # noqa: F401
        return  # already present
    except ImportError:
        pass

    import antenv
    from trn_agent_boot.trn_boot import _ntff_profile_via_ctypes

    mod = types.ModuleType("antenv.axon_hooks")
    _state = {"hook": None}

    def set_axon_ntff_profile_hook(hook):
        _state["hook"] = hook

    def get_axon_ntff_profile_hook():
        return _state["hook"]

    mod.set_axon_ntff_profile_hook = set_axon_ntff_profile_hook
    mod.get_axon_ntff_profile_hook = get_axon_ntff_profile_hook
    sys.modules["antenv.axon_hooks"] = mod
    antenv.axon_hooks = mod
    set_axon_ntff_profile_hook(
        _ntff_profile_via_ctypes("/opt/axon/libaxon_pjrt.so"))


def _run(in_maps, trace=False):
    nc = _build()
    if trace:
        _install_ntff_hook()
    return run_bass_kernel_spmd(nc, in_maps, core_ids=list(range(N_CORES)),
                                trace=trace)


def kernel(x, weights, masks, biases, indices):
    in_maps = _prep_inputs(x, weights, masks, biases)
    res = _run(in_maps)
    out = np.concatenate(
        [np.asarray(res.results[c]["out"][0], dtype=np.float32)
         for c in range(N_CORES)])
    return out


def bench(x, weights, masks, biases, indices):
    """Traced run: returns (output, exec_time_ns)."""
    in_maps = _prep_inputs(x, weights, masks, biases)
    res = _run(in_maps, trace=True)
    out = np.concatenate(
        [np.asarray(res.results[c]["out"][0], dtype=np.float32)
         for c in range(N_CORES)])
    return out, res.exec_time_ns
